# revision 38
# baseline (speedup 1.0000x reference)
"""Bahdanau-style attention scores kernel for Trainium2 (8 NeuronCores).

Reference computation (B=32, S=2048, ENC_H=512, DEC_H=1024):
    W_s = attn_w[:, :1024]; W_e = attn_w[:, 1024:]
    proj_s = s @ W_s.T                      # [B, 1024]
    proj_e = enc @ W_e.T                    # [B, S, 1024]
    scores = tanh(proj_s[:, None] + proj_e) @ v_w.T   # [B, S]
    out = softmax(scores, axis=1)

Strategy: data-parallel over batch (4 batches per core), no collectives.

Layout keeps the hidden dim h on PSUM partitions for the main matmul
    projT[h, s] = sum_e W_eT[e, h] * encT[e, s]
so the per-batch proj_s bias is a per-partition scalar fused into the ACT
tanh. proj_s is precomputed on the host (tiny), enc is pre-cast to bf16
on the host, the weight arrives as 16 half-row descriptors with 1KB
contiguous runs, and the first two PSUM accumulation groups run
ec-outer so the PE starts ~1us after the DMA rings open instead of
waiting for the whole weight.

The v-dot (scores = v . tanh): all 32 M=1 matmuls of one batch form one
contiguous col-group-tiled region (tile_position (0,32j)), so the four
XBUS streams pipeline at the full-array rate and the full<->col-group
drain penalty (~0.6us) is paid once per batch instead of per matmul
group. The four partial rows of each s-block (PSUM partitions
0/32/64/96) are collapsed by a zero-padded full-array matmul against a
4-hot indicator matrix (full-array so it slots into the main stream
with no transition cost). All cross-engine consumers are emitted one
main-MM group late (software pipelining) so the PE queue never waits on
ACT/DVE results.
"""

import numpy as np
import ml_dtypes

import concourse.bass as bass
import concourse.tile as tile
from concourse import mybir
from concourse.bass_utils import run_bass_kernel_spmd

N_CORES = 8
B, S = 32, 2048
E = 1024  # 2*ENC_H, contraction dim of the big matmul
H = 1024  # DEC_H, hidden dim of tanh
D = 1024  # DEC_H, contraction dim of proj_s
BPC = B // N_CORES  # batches per core
P = 128
EC, HC = E // P, H // P
SBLK = 512
NSB = S // SBLK
HH = H // 2  # weight DMA half-row

F32 = mybir.dt.float32
BF16 = mybir.dt.bfloat16
NP_BF16 = ml_dtypes.bfloat16

_cache = {}


def _split_multiwaits(nc):
    """Walrus in this toolchain rejects instructions carrying more than one
    semaphore wait ("Too many sync wait commands"). Engine queues dispatch in
    order, so moving the extra waits onto same-engine NoOps just before the
    instruction is semantically identical."""
    for fn in nc.m.functions:
        for blk in fn.blocks:
            out = []
            for inst in blk.instructions:
                si = inst.sync_info
                waits = list(si.on_wait) if si is not None and si.on_wait else []
                if len(waits) > 1:
                    for i, w in enumerate(waits[:-1]):
                        out.append(
                            mybir.InstNoOp(
                                name=f"{inst.name}-w{i}",
                                engine=inst.engine,
                                sync_info=mybir.SyncInfo(on_wait=[w], on_update=[]),
                                bass_nofuse=True,
                            )
                        )
                    si.on_wait = [waits[-1]]
                    inst.sync_info = si
                out.append(inst)
            try:
                blk.instructions = out
            except Exception:
                blk.set_instructions(out)


def _dedup_ldweights(nc):
    """Drop Ldweights that reload identical array state as the previous one
    (carrying their waits onto the next PE instruction)."""
    ndrop = 0
    for fn in nc.m.functions:
        for blk in fn.blocks:
            out = []
            loaded = None
            pending_waits = []
            for inst in blk.instructions:
                if getattr(inst, "engine", None) != mybir.EngineType.PE:
                    out.append(inst)
                    continue
                if pending_waits:
                    si = inst.sync_info or mybir.SyncInfo(on_wait=[], on_update=[])
                    si.on_wait = list(si.on_wait) + pending_waits
                    inst.sync_info = si
                    pending_waits = []
                if isinstance(inst, mybir.InstLdweights):
                    ap = inst.ins[0]
                    key = (
                        ap.memref,
                        ap.offset,
                        str(ap.ap),
                        str(ap.dtype),
                        str(getattr(inst, "tile_position", None)),
                    )
                    if key == loaded:
                        si = inst.sync_info
                        if si is not None and si.on_wait:
                            pending_waits = list(si.on_wait)
                        if si is not None and si.on_update:
                            out.append(inst)
                            continue
                        ndrop += 1
                        continue
                    loaded = key
                elif isinstance(inst, mybir.InstMatmult):
                    pass  # matmuls stream against loaded weights
                else:
                    loaded = None  # unknown PE instruction: be conservative
                out.append(inst)
            assert not pending_waits
            try:
                blk.instructions = out
            except Exception:
                blk.set_instructions(out)
    return ndrop


def _build_bass(post=True):
    nc = bass.Bass()
    enc_t = nc.dram_tensor("enc_t", [BPC, EC, P, S], BF16, kind="ExternalInput")
    # weight half-rows: [half, ec, p(e), 512(h)] so every descriptor writes
    # 1KB contiguous per partition
    w_t = nc.dram_tensor("w_t", [2, EC, P, HH], BF16, kind="ExternalInput")
    projs_t = nc.dram_tensor("projs_t", [P, HC, BPC], F32, kind="ExternalInput")
    v_t = nc.dram_tensor("v_t", [P, HC, 1], BF16, kind="ExternalInput")
    ind_t = nc.dram_tensor("ind_t", [P, P], BF16, kind="ExternalInput")
    out = nc.dram_tensor("out", [BPC, S], F32, kind="ExternalOutput")

    Tanh = mybir.ActivationFunctionType.Tanh
    Exp = mybir.ActivationFunctionType.Exp

    with tile.TileContext(nc) as tc:
        with (
            tc.tile_pool(name="consts", bufs=1) as consts,
            tc.tile_pool(name="enc", bufs=2) as enc_pool,
            tc.tile_pool(name="tanh", bufs=38) as tanh_pool,
            tc.tile_pool(name="qcopy", bufs=8) as qcopy_pool,
            tc.tile_pool(name="rows", bufs=2) as row_pool,
            tc.tile_pool(name="mmps", bufs=2, space="PSUM") as mm_psum,
            tc.tile_pool(name="quad", bufs=4, space="PSUM") as quad_psum,
            tc.tile_pool(name="cps", bufs=2, space="PSUM") as c_psum,
        ):
            # DMA queue plan (first use decides the critical path):
            #   sync ring:   16 weight half-row descriptors
            #   gpsimd SWDGE: even-ec first s-block slices, then the rest of
            #                the enc stream
            #   scalar ring: projs (needed by the first tanh), odd-ec first
            #                s-block slices, v, indicator
            w_sb = consts.tile([P, EC, H], BF16)
            for half in range(2):
                for ec in range(EC):
                    nc.sync.dma_start(
                        out=w_sb[:, ec, half * HH : (half + 1) * HH],
                        in_=w_t[half, ec],
                    )

            projs_sb = consts.tile([P, HC, BPC], F32)
            nc.scalar.dma_start(out=projs_sb[:], in_=projs_t[:])

            quads = []
            for _ in range(NSB):
                q = quad_psum.tile([P, SBLK], F32, tag="quad")
                quads.append(q)

            # HAM warmup: the PE clock-gate only opens to 2.4 GHz after
            # ~3.4us of sustained activity. Run dummy matmuls on a zeroed
            # scratch tile during the prologue DMA wait so the first real
            # matmuls start warm. Their garbage lands in quads[0], which the
            # memset below re-zeroes (WAW-ordered).
            warm = consts.tile([P, SBLK], BF16)
            nc.vector.memset(warm, 0.0)
            for _ in range(7):
                nc.tensor.matmul(
                    warm_ps := quads[0],
                    warm[:, 0:P],
                    warm,
                    start=True,
                    stop=True,
                    skip_group_check=True,
                )

            for q in quads:
                # The collapse matmul reads all 128 partitions but the
                # v-rounds only ever write 4; zero once so stale PSUM
                # contents never reach the indicator dot.
                nc.vector.memset(q, 0.0)

            # Deferred closures, emitted one main-MM group later so the PE
            # queue never reaches an instruction whose producer (ACT tanh or
            # DVE copy) hasn't had a full group (~1.7us) to finish.
            slots = {}

            def emit_slot(key):
                fns = slots.pop(key, None)
                if fns:
                    for fn in fns:
                        fn()

            def defer(key, fn):
                slots.setdefault(key, []).append(fn)

            v_sb = None
            ind_sb = None

            def dma_enc(b, anchor=None):
                encT = enc_pool.tile([P, EC, S], BF16)
                if b == 0:
                    # Only the first s-block; the bulk stream is emitted after
                    # WAW throttle anchors so its HBM traffic cannot starve
                    # the weight descriptors during the prologue.
                    for ec in range(EC):
                        q = nc.gpsimd if ec % 2 == 0 else nc.scalar
                        q.dma_start(
                            out=encT[:, ec, 0:SBLK], in_=enc_t[0, ec, :, 0:SBLK]
                        )
                else:
                    # WAW anchors tied to the previous batch's progress keep
                    # the scheduler from hoisting these 4MB streams into the
                    # prologue window.
                    for ec in range(EC):
                        nc.vector.tensor_copy(
                            encT[0:1, ec, 0:1], anchor[0:1, ec : ec + 1]
                        )
                    for ec in range(EC):
                        nc.gpsimd.dma_start(out=encT[:, ec, :], in_=enc_t[b, ec])
                return encT

            prev_ths_b = None
            for b in range(BPC):
                encT = dma_enc(b, anchor=None if b == 0 else prev_ths_b[1][0])
                if b == 0:
                    # v / indicator consts ride the scalar ring after the
                    # first s-block's enc slices (not needed until ~15us).
                    v_sb = consts.tile([P, HC, 1], BF16)
                    nc.scalar.dma_start(out=v_sb[:], in_=v_t[:])
                    ind_sb = consts.tile([P, P], BF16)
                    nc.scalar.dma_start(out=ind_sb[:], in_=ind_t[:])

                exp_row = row_pool.tile([1, S], F32, tag="exp_row")
                sums = row_pool.tile([1, NSB], F32, tag="sums")
                ths_b = []
                qbox = {}

                def v_region(sbs, ths_b=ths_b):
                    for r in range(2):
                        for sb in sbs:
                            for j in range(4):
                                hc = 4 * r + j
                                nc.tensor.matmul(
                                    quads[sb][32 * j : 32 * j + 1, :],
                                    v_sb[:, hc, :],
                                    ths_b[sb][hc],
                                    start=(r == 0),
                                    stop=(r == 1),
                                    tile_position=(0, 32 * j),
                                    skip_group_check=True,
                                )

                def qcopy_sb(sb, qbox=qbox):
                    qc = qcopy_pool.tile([P, SBLK], BF16, tag="qcopy")
                    nc.vector.tensor_copy(qc, quads[sb])
                    qbox[sb] = qc

                def collapse_sb(sb, qbox=qbox, exp_row=exp_row, sums=sums):
                    sc = c_psum.tile([P, SBLK], F32, tag="cps")
                    nc.tensor.matmul(sc, ind_sb[:], qbox[sb], start=True, stop=True)
                    nc.scalar.activation(
                        exp_row[:, sb * SBLK : (sb + 1) * SBLK],
                        sc[0:1, :],
                        Exp,
                        accum_out=sums[:, sb : sb + 1],
                    )

                def batch_finale(b=b, exp_row=exp_row, sums=sums):
                    tot = row_pool.tile([1, 1], F32, tag="tot")
                    nc.vector.reduce_sum(tot, sums, axis=mybir.AxisListType.X)
                    rtot = row_pool.tile([1, 1], F32, tag="rtot")
                    nc.vector.reciprocal(rtot, tot)
                    out_row = row_pool.tile([1, S], F32, tag="out_row")
                    half = S // 2
                    nc.vector.tensor_scalar_mul(
                        out_row[:, 0:half], exp_row[:, 0:half], rtot
                    )
                    nc.sync.dma_start(
                        out=out[b : b + 1, 0:half], in_=out_row[:, 0:half]
                    )
                    nc.vector.tensor_scalar_mul(
                        out_row[:, half:S], exp_row[:, half:S], rtot
                    )
                    nc.sync.dma_start(
                        out=out[b : b + 1, half:S], in_=out_row[:, half:S]
                    )

                if b == BPC - 1:
                    # Last batch: s-blocks 0-2 collapse inside this batch's
                    # own main stream (their tanhs are long done); only
                    # s-block 3's short chain trails the final main matmul.
                    defer((b, 26), lambda: v_region(range(3)))
                    defer((b, 27), lambda: qcopy_sb(0))
                    defer((b, 28), lambda: qcopy_sb(1))
                    defer((b, 28), lambda: collapse_sb(0))
                    defer((b, 29), lambda: qcopy_sb(2))
                    defer((b, 29), lambda: collapse_sb(1))
                    defer((b, 30), lambda: collapse_sb(2))
                    defer(("tail", 0), lambda: v_region([3]))
                    defer(("tail", 0), lambda: qcopy_sb(3))
                    defer(("tail", 0), lambda: collapse_sb(3))
                    defer(("tail", 0), batch_finale)

                gi = 0  # main-MM group counter within this batch
                for sb in range(NSB):
                    ths = []
                    if b == 0 and sb == 0:
                        # ec-outer phase for the first two groups: banks from
                        # the (idle) collapse pool accumulate as the weight /
                        # enc descriptors land, so the PE starts on the first
                        # descriptor instead of the eighth; the ec-inner
                        # groups hc2/hc3 then run at full rate on resident h0
                        # data while the h1 half of the weight streams in.
                        ph = [
                            c_psum.tile([P, SBLK], F32, tag="cps", name=f"ph{k}")
                            for k in range(2)
                        ]
                        for ec in range(EC):
                            for hc in range(2):
                                nc.tensor.matmul(
                                    ph[hc],
                                    w_sb[:, ec, hc * P : (hc + 1) * P],
                                    encT[:, ec, 0:SBLK],
                                    start=(ec == 0),
                                    stop=(ec == EC - 1),
                                    skip_group_check=True,
                                )
                        for hc in range(2):
                            th = tanh_pool.tile([P, SBLK], BF16, tag="tanh")
                            nc.scalar.activation(
                                th, ph[hc], Tanh, bias=projs_sb[:, hc, 0:1]
                            )
                            ths.append(th)
                        hc_range = range(2, HC)
                    else:
                        hc_range = range(HC)
                    for hc in hc_range:
                        mm_ps = mm_psum.tile([P, SBLK], F32, tag="mmps")
                        for ec in range(EC):
                            nc.tensor.matmul(
                                mm_ps,
                                w_sb[:, ec, hc * P : (hc + 1) * P],
                                encT[:, ec, sb * SBLK : (sb + 1) * SBLK],
                                start=(ec == 0),
                                stop=(ec == EC - 1),
                            )
                        emit_slot((b, gi))
                        gi += 1
                        th = tanh_pool.tile([P, SBLK], BF16, tag="tanh")
                        nc.scalar.activation(
                            th, mm_ps, Tanh, bias=projs_sb[:, hc, b : b + 1]
                        )
                        ths.append(th)
                        if b == 0 and sb == 0 and hc == 2:
                            # Throttle the bulk enc stream behind tanh(hc2)
                            # via WAW anchors: one stale write into every
                            # descriptor's destination region, so the Tile
                            # scheduler cannot hoist any of the transfers
                            # into the prologue window where they would
                            # starve the weight stream.
                            for ec in range(EC):
                                nc.vector.tensor_copy(
                                    encT[0:1, ec, SBLK : SBLK + 1],
                                    th[0:1, ec : ec + 1],
                                )
                            for ec in range(EC):
                                nc.gpsimd.dma_start(
                                    out=encT[:, ec, SBLK:S],
                                    in_=enc_t[0, ec, :, SBLK:S],
                                )
                    ths_b.append(ths)

                prev_ths_b = ths_b

                if b < BPC - 1:
                    # schedule this batch's postlude into the next batch's
                    # main-MM stream (group index g of batch b+1)
                    nb = b + 1
                    defer((nb, 1), lambda vr=v_region: vr(range(NSB)))
                    for i in range(NSB):
                        defer((nb, 2 + i), (lambda sb=i, f=qcopy_sb: f(sb)))
                    for i in range(NSB):
                        defer((nb, 3 + i), (lambda sb=i, f=collapse_sb: f(sb)))
                    defer((nb, 7), batch_finale)

            emit_slot(("tail", 0))

    if post:
        _dedup_ldweights(nc)
        _split_multiwaits(nc)
    return nc


def _prep_inputs(s, encoder_outputs, attn_w, v_w):
    s = np.asarray(s, dtype=np.float32)
    enc = np.asarray(encoder_outputs, dtype=np.float32)
    attn_w = np.asarray(attn_w, dtype=np.float32)
    v_w = np.asarray(v_w, dtype=np.float32)

    W_s = attn_w[:, :D]  # [H, D]
    W_e = attn_w[:, D:]  # [H, E]
    W_eT = np.ascontiguousarray(W_e.T)  # [E, H]
    # [2 halves, EC, P, 512]: descriptor (half, ec) is [128, 512] with 1KB
    # contiguous runs on both sides
    w_t = np.ascontiguousarray(
        W_eT.reshape(EC, P, 2, HH).transpose(2, 0, 1, 3)
    ).astype(NP_BF16)

    v_t = np.ascontiguousarray(v_w.reshape(HC, P).T).reshape(P, HC, 1).astype(NP_BF16)

    # indicator matrices: matrix k selects partitions {0,32,64,96} into
    # column k, all other columns zero -> the full-array collapse matmul for
    # s-block k writes its score row into PSUM row k and accumulates zeros
    # into every other row of the shared bank
    ind = np.zeros((P, P), dtype=NP_BF16)
    ind[[0, 32, 64, 96], 0] = 1.0

    projs = s @ W_s.T  # [B, H] fp32 on host (tiny)

    in_maps = []
    for c in range(N_CORES):
        lo, hi = c * BPC, (c + 1) * BPC
        enc_c = np.ascontiguousarray(enc[lo:hi].transpose(0, 2, 1)).astype(NP_BF16)
        enc_c = enc_c.reshape(BPC, EC, P, S)
        projs_c = np.ascontiguousarray(
            projs[lo:hi].T.reshape(HC, P, BPC).transpose(1, 0, 2)
        ).astype(np.float32)
        in_maps.append(
            {
                "enc_t": enc_c,
                "w_t": w_t,
                "projs_t": projs_c,
                "v_t": v_t,
                "ind_t": ind,
            }
        )
    return in_maps


def _run(s, encoder_outputs, attn_w, v_w, trace=False):
    if "nc" not in _cache:
        _cache["nc"] = _build_bass()
    nc = _cache["nc"]
    in_maps = _prep_inputs(s, encoder_outputs, attn_w, v_w)
    res = run_bass_kernel_spmd(nc, in_maps, list(range(N_CORES)), trace=trace)
    out = np.concatenate([res.results[c]["out"] for c in range(N_CORES)], axis=0)
    return out.astype(np.float32), res


def kernel(s, encoder_outputs, attn_w, v_w):
    out, _ = _run(s, encoder_outputs, attn_w, v_w, trace=False)
    return out
